# revision 1
# baseline (speedup 1.0000x reference)
"""Block2D shifted-window attention kernel for Trainium2 (8 NeuronCores).

Strategy: the (B=2, 64x64, 2048) input is cyclically shifted and split into
8 independent 32x32 spatial blocks of 1024 tokens each -- exactly one block
per core (data-parallel over the b*bnx*bny block axis; projection weights
replicated).  Each core computes, entirely on-chip in bf16 (fp32 PSUM accum):

  qT = Wq^T @ x^T           [2048, 1024]  (odim on partitions)
  kT = Wk^T @ x^T           [2048, 1024]
  v  = x @ Wv               [1024, 2048]  (tokens on partitions)
  per head h (64-dim, two heads packed per 128-partition tile):
     sT = k_h @ q_h^T       [1024k, 1024q]   (K=64 row-tiled matmuls, 2 heads
                                              concurrent in PE row groups)
     pT = exp(sT / 8)       (softmax without max-subtraction: |s/8| < ~6)
     oT_h = v_h^T @ pT      [64, 1024q]  col-tiled: even head -> psum rows
                            0:64, odd head -> rows 64:128 (concurrent)
     rowsum_h = 1^T @ pT    via M=1 matmuls into psum rows 0 / 32
     oT_h /= rowsum_h       (vector recip + small broadcast matmul)
  out = o^T.T @ Wo          [1024, 2048] fp32 -> HBM

The cyclic shift / block split / unsplit are pure data movement and are folded
into the host-side shard/gather step.
"""

import numpy as np
import ml_dtypes

import concourse.bacc as bacc
import concourse.mybir as mybir
import concourse.tile as tile
from concourse import bass_utils
from concourse.tile import add_dep_helper

HID = 2048
NH = 32
HD = 64
BSH = BSW = 32
SH = SW = 2
P = 128
TOK = 1024           # tokens per block (one core)
KK = HID // P        # 16 contraction tiles
BF16 = mybir.dt.bfloat16
F32 = mybir.dt.float32
BF = ml_dtypes.bfloat16

_NC_CACHE = None
_KVERSION = 4   # bump on every kernel change: defeats shape-keyed NEFF cache


def _emit(tc, nc, xt_d, wq_d, wk_d, wv_d, wo_d, esel_d, ones_d, out_d,
          skip_attention=False, evac_engine="scalar"):
    from contextlib import ExitStack

    if evac_engine == "vector":
        def _evac(out, in_):
            nc.vector.tensor_copy(out, in_)
    else:
        def _evac(out, in_):
            nc.scalar.copy(out, in_)

    with ExitStack() as ctx:
        constp = ctx.enter_context(tc.tile_pool(name="constp", bufs=1))
        xtp = ctx.enter_context(tc.tile_pool(name="xtp", bufs=1))
        vp = ctx.enter_context(tc.tile_pool(name="vp", bufs=1))
        otp = ctx.enter_context(tc.tile_pool(name="otp", bufs=1))
        stgp = ctx.enter_context(tc.tile_pool(name="stgp", bufs=2))

        # ---- constants ----
        esel0 = constp.tile([P, 2 * P], F32)
        nc.sync.dma_start(out=esel0, in_=esel_d.ap())
        ones0 = constp.tile([P, 1], BF16)
        nc.sync.dma_start(out=ones0, in_=ones_d.ap())
        # route consts through ACT so later matmul deps on them are implied
        esel_sb = constp.tile([P, 2 * P], F32)
        nc.scalar.copy(esel_sb, esel0)
        ones_sb = constp.tile([P, 1], BF16)
        nc.scalar.copy(ones_sb, ones0)

        # ---- x^T, resident all kernel ----
        xt_sb = xtp.tile([P, KK * TOK], BF16)
        nc.sync.dma_start(out=xt_sb, in_=xt_d.ap())

        # persistent tiles
        v_sb = []
        for t in range(8):
            v_t = vp.tile([P, HID], BF16, name=f"v{t}", tag=f"v{t}")
            v_sb.append(v_t)
        oT = []
        for j in range(16):
            o_j = otp.tile([P, TOK], BF16, name=f"oT{j}", tag=f"oT{j}")
            oT.append(o_j)

        with ExitStack() as phase1:
            wqkp = phase1.enter_context(tc.tile_pool(name="wqkp", bufs=3))
            wvp = phase1.enter_context(tc.tile_pool(name="wvp", bufs=2))
            qkp = phase1.enter_context(tc.tile_pool(name="qkp", bufs=6))
            pp = phase1.enter_context(tc.tile_pool(name="pp", bufs=5))
            psproj = phase1.enter_context(
                tc.tile_pool(name="psproj", bufs=2, space="PSUM"))
            pss = phase1.enter_context(
                tc.tile_pool(name="pss", bufs=3, space="PSUM"))
            pso = phase1.enter_context(
                tc.tile_pool(name="pso", bufs=2, space="PSUM"))
            psrs = phase1.enter_context(
                tc.tile_pool(name="psrs", bufs=1, space="PSUM"))

            # warmup matmul: makes PE wait on the xt DMA queue once, so every
            # later matmul reading xt has that dep implied (walrus allows only
            # one fresh sem-wait per matmul).
            dps = psproj.tile([P, 512], F32, tag="proj")
            nc.tensor.matmul(dps[0:1, 0:1], xt_sb[:, 0:1], xt_sb[:, 0:1],
                             start=True, stop=True)

            qT = {}
            kT = {}
            for n in range(4):          # output-dim chunk of 512 (4 m-tiles)
                for m in range(4 * n, 4 * n + 4):
                    # ---- qT[m] = Wq[:, m-tile]^T @ x^T ----
                    wqm = wqkp.tile([P, KK * P], BF16, tag="wq")
                    nc.sync.dma_start(
                        out=wqm, in_=wq_d.ap()[:, m * 2048:(m + 1) * 2048])
                    qps = {}
                    for half in range(2):
                        q_ps = psproj.tile([P, 512], F32, tag="proj")
                        for kk in range(KK):
                            nc.tensor.matmul(
                                q_ps,
                                wqm[:, kk * P:(kk + 1) * P],
                                xt_sb[:, kk * TOK + half * 512:
                                      kk * TOK + (half + 1) * 512],
                                start=(kk == 0), stop=(kk == KK - 1))
                        qps[half] = q_ps
                    qTm = qkp.tile([P, TOK], BF16, tag="qT")
                    for half in range(2):
                        _evac(qTm[:, half * 512:(half + 1) * 512], qps[half])
                    qT[m] = qTm

                    # ---- kT[m] ----
                    wkm = wqkp.tile([P, KK * P], BF16, tag="wk")
                    nc.sync.dma_start(
                        out=wkm, in_=wk_d.ap()[:, m * 2048:(m + 1) * 2048])
                    kps = {}
                    for half in range(2):
                        k_ps = psproj.tile([P, 512], F32, tag="proj")
                        for kk in range(KK):
                            nc.tensor.matmul(
                                k_ps,
                                wkm[:, kk * P:(kk + 1) * P],
                                xt_sb[:, kk * TOK + half * 512:
                                      kk * TOK + (half + 1) * 512],
                                start=(kk == 0), stop=(kk == KK - 1))
                        kps[half] = k_ps
                    kTm = qkp.tile([P, TOK], BF16, tag="kT")
                    for half in range(2):
                        _evac(kTm[:, half * 512:(half + 1) * 512], kps[half])
                    kT[m] = kTm

                # ---- v[:, n-chunk] = x @ Wv[:, n-chunk] ----
                wvn = wvp.tile([P, KK * 512], BF16, tag="wv")
                nc.sync.dma_start(
                    out=wvn, in_=wv_d.ap()[:, n * 8192:(n + 1) * 8192])
                for t in range(8):
                    v_ps = psproj.tile([P, 512], F32, tag="proj")
                    for kk in range(KK):
                        nc.tensor.matmul(
                            v_ps,
                            xt_sb[:, kk * TOK + t * P:kk * TOK + (t + 1) * P],
                            wvn[:, kk * 512:(kk + 1) * 512],
                            start=(kk == 0), stop=(kk == KK - 1))
                    _evac(v_sb[t][:, n * 512:(n + 1) * 512], v_ps)

                # ---- attention for head pairs of this chunk ----
                if skip_attention:
                    for j in range(4 * n, 4 * n + 4):
                        nc.scalar.copy(oT[j], xt_sb[:, 0:TOK])
                    continue
                for j in range(4 * n, 4 * n + 4):
                    hA, hB = 2 * j, 2 * j + 1
                    rs_j = psrs.tile([P, 512], F32, tag="rs")
                    o_q = {qb: pso.tile([P, 512], F32, tag="o", name=f"o_q{qb}")
                           for qb in range(2)}
                    o_prev = {0: None, 1: None}
                    rs_prev = {0: None, 1: None}
                    # the two qb streams are interleaved per kb so PE and ACT
                    # always have independent work in flight
                    for kb in range(8):
                        for qb in range(2):
                            rA, rB = 64 * qb, 64 * qb + 32
                            sA = pss.tile([P, 512], F32, tag="s")
                            nc.tensor.matmul(
                                sA,
                                kT[j][0:64, kb * P:(kb + 1) * P],
                                qT[j][0:64, qb * 512:(qb + 1) * 512],
                                start=True, stop=True)
                            sB = pss.tile([P, 512], F32, tag="s")
                            nc.tensor.matmul(
                                sB,
                                kT[j][64:128, kb * P:(kb + 1) * P],
                                qT[j][64:128, qb * 512:(qb + 1) * 512],
                                start=True, stop=True)
                            pa = pp.tile([P, 512], BF16, tag="pa")
                            nc.scalar.activation(
                                pa, sA, mybir.ActivationFunctionType.Exp,
                                scale=0.125)
                            pb = pp.tile([P, 512], BF16, tag="pb")
                            nc.scalar.activation(
                                pb, sB, mybir.ActivationFunctionType.Exp,
                                scale=0.125)
                            # one accumulation group per PSUM bank row-range:
                            # start on the first matmul of the range, stop on
                            # the last; chain same-bank groups in order
                            oa = nc.tensor.matmul(
                                o_q[qb][0:64, :],
                                v_sb[kb][:, hA * 64:(hA + 1) * 64], pa,
                                start=(kb == 0), stop=(kb == 7))
                            if o_prev[qb] is not None:
                                add_dep_helper(oa.ins, o_prev[qb].ins,
                                               sync=False,
                                               reason="psum group order")
                            ob = nc.tensor.matmul(
                                o_q[qb][64:128, :],
                                v_sb[kb][:, hB * 64:(hB + 1) * 64], pb,
                                start=(kb == 0), stop=(kb == 7),
                                skip_group_check=True)
                            add_dep_helper(ob.ins, oa.ins, sync=False,
                                           reason="psum group order")
                            o_prev[qb] = ob
                            ra = nc.tensor.matmul(
                                rs_j[rA:rA + 1, :], ones_sb, pa,
                                start=(kb == 0), stop=(kb == 7),
                                skip_group_check=(rA != 0),
                                tile_position=(0, rA))
                            if rs_prev[qb] is not None:
                                add_dep_helper(ra.ins, rs_prev[qb].ins,
                                               sync=False,
                                               reason="psum group order")
                            rb = nc.tensor.matmul(
                                rs_j[rB:rB + 1, :], ones_sb, pb,
                                start=(kb == 0), stop=(kb == 7),
                                skip_group_check=True,
                                tile_position=(0, rB))
                            add_dep_helper(rb.ins, ra.ins, sync=False,
                                           reason="psum group order")
                            rs_prev[qb] = rb
                    for qb in range(2):
                        _evac(oT[j][0:64, qb * 512:(qb + 1) * 512],
                              o_q[qb][0:64, :])
                        _evac(oT[j][64:128, qb * 512:(qb + 1) * 512],
                              o_q[qb][64:128, :])
                    # softmax denominators -> staging rows 0/32 (qb0) 64/96
                    # (qb1); fill with 1.0 on ACT (Copy: out = in*0 + 1) so
                    # junk rows stay finite through reciprocal
                    stg = stgp.tile([P, 512], F32, tag="stg")
                    nc.scalar.activation(
                        stg, xt_sb[:, 0:512],
                        mybir.ActivationFunctionType.Copy,
                        bias=1.0, scale=0.0)
                    for r in (0, 32, 64, 96):
                        nc.scalar.copy(stg[r:r + 1, :], rs_j[r:r + 1, :])
                    nc.vector.reciprocal(stg, stg)
                    for qb in range(2):
                        bc = pss.tile([P, 512], F32, tag="s")
                        nc.tensor.matmul(
                            bc, esel_sb[:, qb * P:(qb + 1) * P], stg,
                            start=True, stop=True)
                        nc.vector.tensor_mul(
                            out=oT[j][:, qb * 512:(qb + 1) * 512],
                            in0=oT[j][:, qb * 512:(qb + 1) * 512],
                            in1=bc)

        # ---- output projection ----
        with ExitStack() as phase2:
            wop = phase2.enter_context(tc.tile_pool(name="wop", bufs=2))
            outstg = phase2.enter_context(tc.tile_pool(name="outstg", bufs=3))
            psout = phase2.enter_context(
                tc.tile_pool(name="psout", bufs=2, space="PSUM"))
            for nn in range(2):
                won = wop.tile([P, 16 * TOK], BF16, tag="wo")
                nc.sync.dma_start(
                    out=won, in_=wo_d.ap()[:, nn * 16384:(nn + 1) * 16384])
                # warmup matmul so the chunk-DMA wait lands on its own inst
                wps = psout.tile([P, 512], F32, tag="out")
                nc.tensor.matmul(wps[0:1, 0:1], won[:, 0:1], won[:, 0:1],
                                 start=True, stop=True)
                for t in range(8):
                    stage = outstg.tile([P, TOK], F32, tag="ostg")
                    for half in range(2):
                        o_acc = psout.tile([P, 512], F32, tag="out")
                        for j in range(16):
                            nc.tensor.matmul(
                                o_acc,
                                oT[j][:, t * P:(t + 1) * P],
                                won[:, j * TOK + half * 512:
                                    j * TOK + (half + 1) * 512],
                                start=(j == 0), stop=(j == 15))
                        _evac(stage[:, half * 512:(half + 1) * 512], o_acc)
                    nc.sync.dma_start(
                        out=out_d.ap()[t * P:(t + 1) * P,
                                       nn * TOK:(nn + 1) * TOK],
                        in_=stage)


def _build(repeat=1, emit=None, sig=0):
    nc = bacc.Bacc("TRN2", target_bir_lowering=False, debug=False)
    xt_d = nc.dram_tensor("xt", (P, KK * TOK), BF16, kind="ExternalInput")
    wq_d = nc.dram_tensor("wq", (P, 16 * 16 * 128), BF16, kind="ExternalInput")
    wk_d = nc.dram_tensor("wk", (P, 16 * 16 * 128), BF16, kind="ExternalInput")
    wv_d = nc.dram_tensor("wv", (P, 4 * 16 * 512), BF16, kind="ExternalInput")
    wo_d = nc.dram_tensor("wo", (P, 2 * 16 * 1024), BF16, kind="ExternalInput")
    esel_d = nc.dram_tensor("esel", (P, 2 * P), F32, kind="ExternalInput")
    ones_d = nc.dram_tensor("ones", (P, 1), BF16, kind="ExternalInput")
    out_d = nc.dram_tensor("out", (TOK, HID), F32, kind="ExternalOutput")
    # extra output whose shape encodes (kernel version, repeat): the NEFF
    # compile cache keys on the program signature only (it ignores the BIR
    # payload), so every distinct kernel build must have a distinct signature
    rtag_d = nc.dram_tensor("rtag", (1, 1024 * _KVERSION + 32 * sig + repeat),
                            F32, kind="ExternalOutput")

    emit_fn = emit if emit is not None else _emit
    with tile.TileContext(nc) as tc:
        for _ in range(repeat):
            emit_fn(tc, nc, xt_d, wq_d, wk_d, wv_d, wo_d, esel_d, ones_d, out_d)
        with tc.tile_pool(name="rtagp", bufs=1) as rtagp:
            rt = rtagp.tile([1, 1024 * _KVERSION + 32 * sig + repeat], F32)
            nc.vector.memset(rt, 1.0)
            nc.sync.dma_start(out=rtag_d.ap(), in_=rt)
    nc.compile()
    return nc


def _get_nc():
    global _NC_CACHE
    if _NC_CACHE is None:
        _NC_CACHE = _build()
    return _NC_CACHE


def _shard_inputs(hidden_states, Wq, Wk, Wv, Wo):
    B = hidden_states.shape[0]
    x2 = hidden_states.reshape(B, 64, 64, HID)
    x2 = np.roll(x2, shift=(-SH, -SW), axis=(1, 2))
    xb = (x2.reshape(B, 2, BSH, 2, BSW, HID)
          .transpose(0, 1, 3, 2, 4, 5)
          .reshape(B * 4, TOK, HID)
          .astype(BF))

    wq_r = np.ascontiguousarray(
        Wq.astype(BF).reshape(16, 128, 16, 128).transpose(1, 2, 0, 3)
        .reshape(128, 32768))
    wk_r = np.ascontiguousarray(
        Wk.astype(BF).reshape(16, 128, 16, 128).transpose(1, 2, 0, 3)
        .reshape(128, 32768))
    wv_r = np.ascontiguousarray(
        Wv.astype(BF).reshape(16, 128, 4, 512).transpose(1, 2, 0, 3)
        .reshape(128, 32768))
    wo_r = np.ascontiguousarray(
        Wo.astype(BF).reshape(16, 128, 2, 1024).transpose(1, 2, 0, 3)
        .reshape(128, 32768))
    esel = np.zeros((P, 2 * P), np.float32)
    esel[0, 0:64] = 1.0          # qb0 even head <- row 0
    esel[32, 64:128] = 1.0       # qb0 odd head  <- row 32
    esel[64, 128 + 0:128 + 64] = 1.0    # qb1 even <- row 64
    esel[96, 128 + 64:128 + 128] = 1.0  # qb1 odd  <- row 96
    ones = np.ones((P, 1), BF)

    in_maps = []
    for c in range(8):
        xt = np.ascontiguousarray(
            xb[c].T.reshape(KK, P, TOK).transpose(1, 0, 2).reshape(P, KK * TOK))
        in_maps.append({
            "xt": xt, "wq": wq_r, "wk": wk_r, "wv": wv_r, "wo": wo_r,
            "esel": esel, "ones": ones,
        })
    return in_maps


def _unshard(outs, B):
    o = np.stack(outs)                       # (8, 1024, 2048)
    o = (o.reshape(B, 2, 2, BSH, BSW, HID)
         .transpose(0, 1, 3, 2, 4, 5)
         .reshape(B, 64, 64, HID))
    o = np.roll(o, shift=(SH, SW), axis=(1, 2))
    return o.reshape(B, 64 * 64, HID)


def kernel(hidden_states, Wq, Wk, Wv, Wo, h_dim=64, w_dim=64, _trace=False):
    hidden_states = np.asarray(hidden_states, dtype=np.float32)
    Wq = np.asarray(Wq, dtype=np.float32)
    Wk = np.asarray(Wk, dtype=np.float32)
    Wv = np.asarray(Wv, dtype=np.float32)
    Wo = np.asarray(Wo, dtype=np.float32)
    B = hidden_states.shape[0]

    nc = _get_nc()
    in_maps = _shard_inputs(hidden_states, Wq, Wk, Wv, Wo)
    res = bass_utils.run_bass_kernel_spmd(
        nc, in_maps, core_ids=list(range(8)), trace=_trace)
    out = _unshard([res.results[c]["out"] for c in range(8)], B)
    if _trace:
        kernel._last_results = res
    return out

